# revision 27
# baseline (speedup 1.0000x reference)
"""Trainium2 Bass kernel for nn_AxialBlock (3-axis axial attention sum).

Problem (hardcoded): x (B=4, C=512, T=16, H=32, W=32) fp32, three axial
MHA blocks (attend along W, H, T; n_head=8, d=64) each with their own
QKVO projections; outputs summed. Output (B, C, T, H, W) fp32.

Sharding: 8 cores = (batch b in 0..3) x (pair index j in 0..1).
  - w-pass / t-pass: tokens split by H-half (j); fully local.
  - h-pass: tokens split by W-half (j); fully local (attention along H
    needs all H at fixed (t, w), so a W split keeps fibers intact).
    Each pass is 8192 tokens per core; all passes are structurally
    identical 8-head attention over 32-token fibers (the t-pass has
    16-token fibers handled by a rank-2 additive mask).

On-device layout: x is channels-first ("x^T", C on partitions). Host
pre-permutes x into three token orders (w-fastest / t-fastest /
h-fastest) so each axial attention acts on 32 consecutive tokens.

Matmul structure: q/k/out projections are emitted in super-tiles of
ST=4 token tiles with the weight chunk as the stationary operand reused
across the 4 sub-tiles (4 PSUM banks accumulate in parallel) — the PE
reloads its stationary every matmul otherwise, and the ~107ns weight
load is NOT hidden. v must be token-partitioned (it is the O^T
stationary), so its projection keeps per-tile stationaries (ts-outer,
2 PSUM banks live).

PE-stream interleave (per super-tile i): scores(st) | v(st+1) | AV(st)
so the v-projection matmuls fill the PE while sub-tile st's softmax
chain runs on Scalar/Vector/GpSimd; at st=3 the NEXT super-tile's
q-projection is emitted before AV(3) (then out-proj(i), then k(i+1)),
so the last softmax gap is filled by independent q matmuls.

Attention per 512-token tile (16 rows x 32 tokens): k is evacuated
parity-split into persistent pre-zeroed "kz" buffers (one head per 64
d-rows) so scores contract over all 128 partitions; one (K=128, M=32,
N=64) matmul per (chunk, row) computes both heads of the chunk at
col-tile (0, 32j). Softmax: exp on ScalarE, reduce+reciprocal on
VectorE, broadcast normalize on GpSimd. The t-pass cross-fiber mask is
a rank-2 matmul (-60 additive) accumulated under the scores before exp.
A -> A^T via the DVE 32x32 block transpose, then DVE copies form a
block-diagonal A^T ("abd"); o^T = V^T @ abd lands feature-partitioned;
then the out-projection (ST=4 weight reuse) and y accumulation: w-pass
writes y + summed bias, t-pass does a strided DRAM read-modify-write
add, h-pass writes its own y_h (bf16) for its disjoint token set.
"""

import contextlib

import ml_dtypes
import numpy as np

import concourse.bass as bass
import concourse.tile as tile
from concourse import bacc, mybir
from concourse.bass_utils import run_bass_kernel_spmd

BF16 = mybir.dt.bfloat16
FP32 = mybir.dt.float32
BF16_NP = np.dtype(ml_dtypes.bfloat16)

B, C, T, H, W = 4, 512, 16, 32, 32
NH, D = 8, 64
HL = H // 2              # per-core H slice (w/t passes)
WL = W // 2              # per-core W slice (h pass)
N_CORES = 8
TOK_LOCAL = T * HL * W   # 8192 tokens owned per core (all passes)
TILE = 512               # tokens per on-chip tile
NCH = C // 128           # 4 partition chunks of the feature dim
ST = 4                   # sub-tiles per super-tile (stationary reuse)
NSUP = TOK_LOCAL // TILE // ST   # 4 super-tiles per pass


def _proj_phase(tc, ps_pool, n_mc, n_kc, lhs_fn, rhs_fn, evac_fn):
    """One ST-wide projection phase: stationary reused across ST sub-tiles.

    lhs_fn(mc, kc) -> stationary AP; rhs_fn(st, kc) -> moving AP;
    evac_fn(mc, st, ps) consumes the finished PSUM tile.
    """
    nc = tc.nc
    for mc in range(n_mc):
        pss = []
        for st in range(ST):
            ps = ps_pool.tile([128, TILE], FP32, tag=f"p{st}", bufs=2,
                              name=f"ps{st}")
            pss.append(ps)
        for kc in range(n_kc):
            for st in range(ST):
                nc.tensor.matmul(
                    pss[st][:],
                    lhsT=lhs_fn(mc, kc),
                    rhs=rhs_fn(st, kc),
                    start=(kc == 0), stop=(kc == n_kc - 1),
                )
        for st in range(ST):
            evac_fn(mc, st, pss[st])


def _v_phase(tc, pools, st, xt, wv_sb):
    """v projection for one sub-tile, token-partitioned (x is the
    stationary), ts-outer so only 2 PSUM banks are live. These dense
    matmuls are the PE gap filler under the softmax chains."""
    nc = tc.nc
    v_pool, ps_pool = pools["v"], pools["ps"]
    v_sb = v_pool.tile([128, NCH, NCH, 128], BF16, tag=f"v{st}", bufs=1,
                       name=f"v{st}")
    v2 = v_sb[:].rearrange("p a b c -> p (a b c)")
    for ts in range(NCH):
        ps = ps_pool.tile([128, TILE], FP32, tag=f"p{ts % 2}", bufs=2,
                          name="psv")
        for kc in range(NCH):
            nc.tensor.matmul(
                ps[:],
                lhsT=xt[:, st, kc, 128 * ts:128 * (ts + 1)],
                rhs=wv_sb[:, kc, :],
                start=(kc == 0), stop=(kc == NCH - 1),
            )
        if ts % 2 == 0:
            nc.scalar.copy(v2[:, 512 * ts:512 * (ts + 1)], ps[:])
        else:
            nc.vector.tensor_copy(v2[:, 512 * ts:512 * (ts + 1)], ps[:])
    return v_sb


def _scores_phase(tc, pools, axis, st, q_sb, kz, tml_sb, tmr_sb):
    """S + softmax chain for one sub-tile; returns the two abd tiles
    (block-diagonal normalized A^T) the AV phase will consume."""
    nc = tc.nc
    a_pool, sm_pool, ps_pool = pools["a"], pools["sm"], pools["ps"]
    GW = NH * 32
    abds = []
    for gg in range(2):
        sps = ps_pool.tile([128, 2 * GW], FP32, tag=f"p{2 + gg}", bufs=2,
                           name="sps")
        base = axis == "t"
        if base:
            nc.tensor.matmul(
                sps[:], lhsT=tml_sb[:], rhs=tmr_sb[:],
                start=True, stop=False, skip_group_check=True,
            )
        nmm = 32
        i_mm = 0
        for gh in range(2):
            g = 2 * gg + gh
            for c in range(NCH):
                for j in range(4):
                    qcol = (g * 4 + j) * 32
                    i_mm += 1
                    nc.tensor.matmul(
                        sps[32 * j:32 * (j + 1),
                            gh * GW + 2 * c * 32:gh * GW + (2 * c + 2) * 32],
                        lhsT=q_sb[:, st, c, qcol:qcol + 32],
                        rhs=kz[:, :, c,
                               (g * 4 + j) * 32:(g * 4 + j) * 32 + 32],
                        tile_position=(0, 32 * j),
                        start=(not base),
                        stop=(base and i_mm == nmm),
                        skip_group_check=True,
                    )
        # softmax over k (free axis)
        a_sb = a_pool.tile([128, 2 * GW], BF16, tag="a")
        nc.scalar.activation(a_sb[:], sps[:],
                             mybir.ActivationFunctionType.Exp)
        a3 = a_sb[:].rearrange("p (n k) -> p n k", n=2 * NH)
        sums = sm_pool.tile([128, 2 * NH], FP32, tag="sums")
        nc.vector.tensor_reduce(
            sums[:], a3, axis=mybir.AxisListType.X, op=mybir.AluOpType.add
        )
        recip = sm_pool.tile([128, 2 * NH], FP32, tag="recip")
        nc.vector.reciprocal(recip[:], sums[:])
        nc.gpsimd.tensor_tensor(
            a3, a3,
            recip[:].unsqueeze(2).broadcast_to((128, 2 * NH, 32)),
            mybir.AluOpType.mult,
        )
        at_sb = a_pool.tile([128, 2 * GW], BF16, tag="at")
        nc.vector.transpose(at_sb[:], a_sb[:])
        abd = pools["abd"][tc._abd_flip]
        tc._abd_flip = (tc._abd_flip + 1) % len(pools["abd"])
        for j in range(4):
            nc.gpsimd.tensor_copy(
                abd[32 * j:32 * (j + 1), 512 * j:512 * (j + 1)],
                at_sb[32 * j:32 * (j + 1), :],
            )
        abds.append(abd)
    return abds


def _av_phase(tc, pools, st, v_sb, abds, ot_sb):
    """O^T = V^T @ A_bd for one sub-tile, chunk-outer."""
    nc = tc.nc
    ps_pool = pools["ps"]
    GW = NH * 32
    for c in range(NCH):
        otp = ps_pool.tile([128, TILE], FP32, name="otp",
                           tag=f"p{c % 2}", bufs=2)
        for g in range(4):
            gh = g % 2
            abd4 = abds[g // 2][:].rearrange("p (j x) -> p j x", j=4)
            for p in range(2):
                s0 = gh * GW + (2 * c + p) * 32
                nc.tensor.matmul(
                    otp[64 * p:64 * (p + 1), g * 128:(g + 1) * 128],
                    lhsT=v_sb[:, g, c, 64 * p:64 * (p + 1)],
                    rhs=abd4[:, :, s0:s0 + 32],
                    tile_position=(0, 64 * p),
                )
        if c % 2 == 0:
            nc.scalar.copy(ot_sb[:, st, c, :], otp[:])
        else:
            nc.vector.tensor_copy(ot_sb[:, st, c, :], otp[:])


def _load_xt(tc, pools, x_ap, sup):
    nc = tc.nc
    xt = pools["xt"].tile([128, ST, NCH, TILE], BF16, tag="xt")
    for st in range(ST):
        for kc in range(NCH):
            nc.sync.dma_start(
                xt[:, st, kc, :],
                x_ap[128 * kc:128 * (kc + 1),
                     (sup * ST + st) * TILE:(sup * ST + st + 1) * TILE])
    return xt


def _q_phase(tc, pools, xt, wq_sb):
    nc = tc.nc
    q_sb = pools["qk"].tile([128, ST, NCH, TILE], BF16, tag="q", bufs=2)

    def q_evac(mc, st, ps):
        if st % 2 == 0:
            nc.scalar.copy(q_sb[:, st, mc, :], ps[:])
        else:
            nc.vector.tensor_copy(q_sb[:, st, mc, :], ps[:])

    _proj_phase(tc, pools["ps"], NCH, NCH,
                lambda mc, kc: wq_sb[:, kc, 128 * mc:128 * (mc + 1)],
                lambda st, kc: xt[:, st, kc, :], q_evac)
    return q_sb


def _build_pass(tc, pools, axis, x_ap, w_aps, y_ap, tml_sb, tmr_sb,
                kz_tiles, preloaded=None, next_prologue=None):
    """One axial pass over the core's 8192 owned tokens; writes its own
    bf16 y output in the pass-local token order (host sums the three)."""
    nc = tc.nc
    wq_sb, wk_sb, wv_sb, wo_sb = w_aps
    ot_pool, y_pool, ps_pool = pools["ot"], pools["y"], pools["ps"]

    def load_xt(sup):
        return _load_xt(tc, pools, x_ap, sup)

    def q_phase(xt):
        return _q_phase(tc, pools, xt, wq_sb)

    def k_phase(xt):
        def k_evac(mc, st, ps):
            kz = kz_tiles[st]
            if mc < 2:
                nc.scalar.copy(kz[0:64, 0, mc, :], ps[0:64, :])
                nc.scalar.copy(kz[64:128, 1, mc, :], ps[64:128, :])
            else:
                nc.vector.tensor_copy(kz[0:64, 0, mc, :], ps[0:64, :])
                nc.vector.tensor_copy(kz[64:128, 1, mc, :], ps[64:128, :])

        _proj_phase(tc, ps_pool, NCH, NCH,
                    lambda mc, kc: wk_sb[:, kc, 128 * mc:128 * (mc + 1)],
                    lambda st, kc: xt[:, st, kc, :], k_evac)

    def o_phase(sup, ot_sb):
        it0 = sup * ST

        def y_evac(mc, st, ps):
            it = it0 + st
            cs = slice(128 * mc, 128 * (mc + 1))
            y_sb = y_pool.tile([128, TILE], BF16, tag="y_sb", bufs=4)
            if mc % 2 == 0:
                nc.scalar.copy(y_sb[:], ps[:])
            else:
                nc.vector.tensor_copy(y_sb[:], ps[:])
            nc.scalar.dma_start(y_ap[cs, it * TILE:(it + 1) * TILE],
                                y_sb[:])

        _proj_phase(tc, ps_pool, NCH, NCH,
                    lambda mc, kc: wo_sb[:, kc, 128 * mc:128 * (mc + 1)],
                    lambda st, kc: ot_sb[:, st, kc, :], y_evac)

    # ---- pass body with PE-stream interleave: scores run 2 sub-tiles
    # ahead of AV (the softmax chain of st hides under scores of st+1);
    # the next super-tile's q projection fills the last chain's gap, and
    # at the pass end the NEXT pass's first q projection does (prologue).
    if preloaded is not None:
        xt, q_sb = preloaded
    else:
        xt = load_xt(0)
        q_sb = q_phase(xt)
    k_phase(xt)
    next_pre = None
    for sup in range(NSUP):
        ot_sb = ot_pool.tile([128, ST, NCH, TILE], BF16, tag="ot", bufs=1)
        xt_next = q_next = None
        v_tiles = {0: _v_phase(tc, pools, 0, xt, wv_sb)}
        abds_by_st = {}
        for st in range(ST):
            abds_by_st[st] = _scores_phase(tc, pools, axis, st, q_sb,
                                           kz_tiles[st], tml_sb, tmr_sb)
            if st < ST - 1:
                v_tiles[st + 1] = _v_phase(tc, pools, st + 1, xt, wv_sb)
            if st >= 1:
                _av_phase(tc, pools, st - 1, v_tiles.pop(st - 1),
                          abds_by_st.pop(st - 1), ot_sb)
        if sup + 1 < NSUP:
            xt_next = load_xt(sup + 1)
            q_next = q_phase(xt_next)
        elif next_prologue is not None:
            next_pre = next_prologue()
        _av_phase(tc, pools, ST - 1, v_tiles.pop(ST - 1),
                  abds_by_st.pop(ST - 1), ot_sb)
        o_phase(sup, ot_sb)
        if sup + 1 < NSUP:
            k_phase(xt_next)
            xt, q_sb = xt_next, q_next
    return next_pre


def build_program():
    """Build + compile the SPMD bass program (same program on all 8 cores)."""
    nc = bacc.Bacc(
        "TRN2", target_bir_lowering=False, debug=False,
        enable_asserts=False, num_devices=N_CORES,
    )

    def din(name, shape, dt=BF16):
        return nc.dram_tensor(name, shape, dt, kind="ExternalInput").ap()

    x_in = {ax: din(f"x_{ax}", (C, TOK_LOCAL)) for ax in ("w", "t", "h")}
    w_in = {}
    for ax in ("w", "t", "h"):
        for nm in ("wq", "wk", "wv", "wo"):
            w_in[f"{nm}_{ax}"] = din(f"{nm}_{ax}", (C, C))
    tml_in = din("tml", (2, 128))
    tmr_in = din("tmr", (2, 512))
    y_aps = {ax: nc.dram_tensor(f"y_{ax}", (C, TOK_LOCAL), BF16,
                                kind="ExternalOutput").ap()
             for ax in ("w", "t", "h")}

    with tile.TileContext(nc) as tc:
        with contextlib.ExitStack() as ctx:
            xt_pool = ctx.enter_context(tc.tile_pool(name="xt", bufs=2))
            w_pool = ctx.enter_context(tc.tile_pool(name="wts", bufs=1))
            qk_pool = ctx.enter_context(tc.tile_pool(name="qk", bufs=2))
            v_pool = ctx.enter_context(tc.tile_pool(name="v", bufs=1))
            a_pool = ctx.enter_context(tc.tile_pool(name="a", bufs=3))
            sm_pool = ctx.enter_context(tc.tile_pool(name="sm", bufs=3))
            ot_pool = ctx.enter_context(tc.tile_pool(name="ot", bufs=1))
            y_pool = ctx.enter_context(tc.tile_pool(name="y", bufs=2))
            ps_pool = ctx.enter_context(tc.tile_pool(name="ps", bufs=1,
                                                     space="PSUM"))
            const_pool = ctx.enter_context(tc.tile_pool(name="const", bufs=1))

            # constants
            tml_sb = const_pool.tile([2, 128], BF16)
            nc.sync.dma_start(tml_sb[:], tml_in[:])
            tmr_sb = const_pool.tile([2, 512], BF16)
            nc.sync.dma_start(tmr_sb[:], tmr_in[:])

            # persistent block-diagonal A^T buffers and parity-split k
            # buffers (one per sub-tile), zeroed once
            abd_tiles = []
            for i in range(4):
                t = const_pool.tile([128, 4 * 512], BF16, name=f"abd{i}")
                nc.gpsimd.memset(t[:], 0.0)
                abd_tiles.append(t)
            tc._abd_flip = 0
            kz_tiles = []
            for i in range(ST):
                t = const_pool.tile([128, 2, NCH, TILE], BF16, name=f"kz{i}")
                nc.gpsimd.memset(t[:], 0.0)
                kz_tiles.append(t)

            pools = {"xt": xt_pool, "qk": qk_pool, "v": v_pool,
                     "a": a_pool, "sm": sm_pool, "ot": ot_pool,
                     "y": y_pool, "ps": ps_pool, "abd": abd_tiles}

            def load_w_one(ax, nm):
                wt = w_pool.tile([128, NCH, C], BF16, tag=nm, name=nm)
                for kc in range(NCH):
                    nc.sync.dma_start(
                        wt[:, kc, :],
                        w_in[f"{nm}_{ax}"][128 * kc:128 * (kc + 1), :],
                    )
                return wt

            axes = ("w", "t", "h")

            def make_prologue(nxt_ax):
                # emitted at the previous pass's last softmax gap: load
                # the next pass's wq + first x super-tile and emit its
                # q projection (independent PE work that fills the gap)
                def prologue():
                    wq_n = load_w_one(nxt_ax, "wq")
                    xt0 = _load_xt(tc, pools, x_in[nxt_ax], 0)
                    q0 = _q_phase(tc, pools, xt0, wq_n)
                    return (wq_n, xt0, q0)
                return prologue

            pre = None
            for i, ax in enumerate(axes):
                if pre is None:
                    wq_sb = load_w_one(ax, "wq")
                    xt0 = _load_xt(tc, pools, x_in[ax], 0)
                    q0 = _q_phase(tc, pools, xt0, wq_sb)
                else:
                    wq_sb, xt0, q0 = pre
                w_aps = [wq_sb] + [load_w_one(ax, nm)
                                   for nm in ("wk", "wv", "wo")]
                nxt = (make_prologue(axes[i + 1])
                       if i + 1 < len(axes) else None)
                pre = _build_pass(tc, pools, ax, x_in[ax], w_aps, y_aps[ax],
                                  tml_sb, tmr_sb, kz_tiles,
                                  preloaded=(xt0, q0), next_prologue=nxt)

    nc.compile()
    return nc


_PROGRAM = None


def _get_program():
    global _PROGRAM
    if _PROGRAM is None:
        _PROGRAM = build_program()
    return _PROGRAM


def make_in_maps(inputs):
    """Host-side shard + layout prep: per-core input dicts."""
    global _BIAS
    x = np.asarray(inputs["x"], np.float32)          # (B, C, T, H, W)
    scale = 1.0 / np.sqrt(D)
    _BIAS = (np.asarray(inputs["bo_w"], np.float32)
             + np.asarray(inputs["bo_h"], np.float32)
             + np.asarray(inputs["bo_t"], np.float32)).reshape(C, 1, 1, 1)

    weights = {}
    for ax in ("w", "t", "h"):
        for nm in ("wq", "wk", "wv", "wo"):
            wm = np.asarray(inputs[f"{nm}_{ax}"], np.float32)
            if nm == "wq":
                wm = wm * scale
            # lhsT layout: (C_in, C_out) = W.T
            weights[f"{nm}_{ax}"] = np.ascontiguousarray(wm.T).astype(BF16_NP)
    # rank-2 additive cross-fiber mask for the t-pass:
    # S += tml.T @ tmr with tml one-hot on the query fiber and tmr = -60 on
    # cross-fiber key columns
    p = np.arange(128) % 32
    tml = np.stack([(p // 16) == e for e in range(2)]).astype(BF16_NP)
    f = np.arange(512) % 32
    tmr = np.stack([np.where((f // 16) != e, -60.0, 0.0) for e in range(2)]
                   ).astype(BF16_NP)

    in_maps = []
    for core in range(N_CORES):
        b, j = divmod(core, 2)
        xb = x[b]                                    # (C, T, H, W)
        xw = xb[:, :, HL * j:HL * (j + 1), :]        # (C, T, HL, W) w-fastest
        xt = np.transpose(xw, (0, 2, 3, 1))          # (C, HL, W, T) t-fastest
        xh = np.transpose(xb, (0, 1, 3, 2))[:, :, WL * j:WL * (j + 1), :]
        # xh: (C, T, WL, H) h-fastest
        m = {
            "x_w": np.ascontiguousarray(xw).reshape(C, TOK_LOCAL).astype(BF16_NP),
            "x_t": np.ascontiguousarray(xt).reshape(C, TOK_LOCAL).astype(BF16_NP),
            "x_h": np.ascontiguousarray(xh).reshape(C, TOK_LOCAL).astype(BF16_NP),
            "tml": tml, "tmr": tmr,
        }
        m.update(weights)
        in_maps.append(m)
    return in_maps


_BIAS = None


def assemble_output(results):
    """Gather per-core y_w/y_t/y_h into (B, C, T, H, W) fp32."""
    out = np.empty((B, C, T, H, W), np.float32)
    bias = (_BIAS if _BIAS is not None else 0.0)
    for b in range(B):
        for j in range(2):
            core = 2 * b + j
            hs = slice(HL * j, HL * (j + 1))
            yw = np.asarray(results[core]["y_w"]).astype(np.float32)
            yt = np.asarray(results[core]["y_t"]).astype(np.float32)
            out[b, :, :, hs, :] = (
                yw.reshape(C, T, HL, W)
                + yt.reshape(C, HL, W, T).transpose(0, 3, 1, 2)
                + bias)
        for j in range(2):
            core = 2 * b + j
            yh = np.asarray(results[core]["y_h"]).astype(np.float32)
            out[b, :, :, :, WL * j:WL * (j + 1)] += yh.reshape(
                C, T, WL, H).transpose(0, 1, 3, 2)
    return out


_RUNNER = None


def _get_runner():
    """Build the sharded PJRT callable once; reuse across kernel() calls."""
    global _RUNNER
    if _RUNNER is not None:
        return _RUNNER
    import jax
    from jax.sharding import Mesh, PartitionSpec
    from jax.experimental.shard_map import shard_map
    from concourse import bass2jax

    nc = _get_program()
    bass2jax.install_neuronx_cc_hook()
    partition_name = (nc.partition_id_tensor.name
                      if nc.partition_id_tensor else None)
    in_names, out_names, out_avals, zero_outs = [], [], [], []
    for alloc in nc.m.functions[0].allocations:
        if not isinstance(alloc, mybir.MemoryLocationSet):
            continue
        name = alloc.memorylocations[0].name
        if alloc.kind == "ExternalInput":
            if name != partition_name:
                in_names.append(name)
        elif alloc.kind == "ExternalOutput":
            out_names.append(name)
            shape = tuple(alloc.tensor_shape)
            dtype = mybir.dt.np(alloc.dtype)
            out_avals.append(jax.core.ShapedArray(shape, dtype))
            zero_outs.append(np.zeros((N_CORES * shape[0], *shape[1:]), dtype))
    n_params = len(in_names)
    all_in_names = list(in_names) + out_names
    if partition_name is not None:
        all_in_names.append(partition_name)

    def _body(*args):
        operands = list(args)
        if partition_name is not None:
            operands.append(bass2jax.partition_id_tensor())
        return tuple(bass2jax._bass_exec_p.bind(
            *operands,
            out_avals=tuple(out_avals),
            in_names=tuple(all_in_names),
            out_names=tuple(out_names),
            lowering_input_output_aliases=(),
            sim_require_finite=True,
            sim_require_nnan=True,
            nc=nc,
        ))

    devices = jax.devices()[:N_CORES]
    mesh = Mesh(np.asarray(devices), ("core",))
    in_specs = (PartitionSpec("core"),) * (n_params + len(out_names))
    out_specs = (PartitionSpec("core"),) * len(out_names)
    fn = jax.jit(shard_map(_body, mesh=mesh, in_specs=in_specs,
                           out_specs=out_specs, check_rep=False))

    def run(in_maps):
        concat_in = [
            np.concatenate([np.asarray(in_maps[c][nm]) for c in range(N_CORES)],
                           axis=0)
            for nm in in_names
        ]
        outs = fn(*concat_in, *zero_outs)
        return [
            {nm: np.asarray(outs[i]).reshape(N_CORES, *out_avals[i].shape)[c]
             for i, nm in enumerate(out_names)}
            for c in range(N_CORES)
        ]

    _RUNNER = run
    return run


def kernel(**inputs) -> np.ndarray:
    run = _get_runner()
    in_maps = make_in_maps(inputs)
    return assemble_output(run(in_maps))


# revision 28
# speedup vs baseline: 1.0693x; 1.0693x over previous
"""Trainium2 Bass kernel for nn_AxialBlock (3-axis axial attention sum).

Problem (hardcoded): x (B=4, C=512, T=16, H=32, W=32) fp32, three axial
MHA blocks (attend along W, H, T; n_head=8, d=64) each with their own
QKVO projections; outputs summed. Output (B, C, T, H, W) fp32.

Sharding: 8 cores = (batch b in 0..3) x (pair index j in 0..1).
  - w-pass / t-pass: tokens split by H-half (j); fully local.
  - h-pass: tokens split by W-half (j); fully local (attention along H
    needs all H at fixed (t, w), so a W split keeps fibers intact).
    Each pass is 8192 tokens per core; all passes are structurally
    identical 8-head attention over 32-token fibers (the t-pass has
    16-token fibers handled by a rank-2 additive mask).

On-device layout: x is channels-first ("x^T", C on partitions). Host
pre-permutes x into three token orders (w-fastest / t-fastest /
h-fastest) so each axial attention acts on 32 consecutive tokens.

Matmul structure: q/k/out projections are emitted in super-tiles of
ST=4 token tiles with the weight chunk as the stationary operand reused
across the 4 sub-tiles (4 PSUM banks accumulate in parallel) — the PE
reloads its stationary every matmul otherwise, and the ~107ns weight
load is NOT hidden. v must be token-partitioned (it is the O^T
stationary), so its projection keeps per-tile stationaries (ts-outer,
2 PSUM banks live).

PE-stream interleave (per super-tile i): scores(st) | v(st+1) | AV(st)
so the v-projection matmuls fill the PE while sub-tile st's softmax
chain runs on Scalar/Vector/GpSimd; at st=3 the NEXT super-tile's
q-projection is emitted before AV(3) (then out-proj(i), then k(i+1)),
so the last softmax gap is filled by independent q matmuls.

Attention per 512-token tile (16 rows x 32 tokens): k is evacuated
parity-split into persistent pre-zeroed "kz" buffers (one head per 64
d-rows) so scores contract over all 128 partitions; one (K=128, M=32,
N=64) matmul per (chunk, row) computes both heads of the chunk at
col-tile (0, 32j). Softmax: exp on ScalarE, reduce+reciprocal on
VectorE, broadcast normalize on GpSimd. The t-pass cross-fiber mask is
a rank-2 matmul (-60 additive) accumulated under the scores before exp.
A -> A^T via the DVE 32x32 block transpose, then DVE copies form a
block-diagonal A^T ("abd"); o^T = V^T @ abd lands feature-partitioned;
then the out-projection (ST=4 weight reuse) and y accumulation: w-pass
writes y + summed bias, t-pass does a strided DRAM read-modify-write
add, h-pass writes its own y_h (bf16) for its disjoint token set.
"""

import contextlib

import ml_dtypes
import numpy as np

import concourse.bass as bass
import concourse.tile as tile
from concourse import bacc, mybir
from concourse.bass_utils import run_bass_kernel_spmd

BF16 = mybir.dt.bfloat16
FP32 = mybir.dt.float32
BF16_NP = np.dtype(ml_dtypes.bfloat16)

B, C, T, H, W = 4, 512, 16, 32, 32
NH, D = 8, 64
HL = H // 2              # per-core H slice (w/t passes)
WL = W // 2              # per-core W slice (h pass)
N_CORES = 8
TOK_LOCAL = T * HL * W   # 8192 tokens owned per core (all passes)
TILE = 512               # tokens per on-chip tile
NCH = C // 128           # 4 partition chunks of the feature dim
ST = 4                   # sub-tiles per super-tile (stationary reuse)
NSUP = TOK_LOCAL // TILE // ST   # 4 super-tiles per pass


def _proj_phase(tc, ps_pool, n_mc, n_kc, lhs_fn, rhs_fn, evac_fn):
    """One ST-wide projection phase: stationary reused across ST sub-tiles.

    lhs_fn(mc, kc) -> stationary AP; rhs_fn(st, kc) -> moving AP;
    evac_fn(mc, st, ps) consumes the finished PSUM tile.
    """
    nc = tc.nc
    for mc in range(n_mc):
        pss = []
        for st in range(ST):
            ps = ps_pool.tile([128, TILE], FP32, tag=f"p{st}", bufs=2,
                              name=f"ps{st}")
            pss.append(ps)
        for kc in range(n_kc):
            for st in range(ST):
                nc.tensor.matmul(
                    pss[st][:],
                    lhsT=lhs_fn(mc, kc),
                    rhs=rhs_fn(st, kc),
                    start=(kc == 0), stop=(kc == n_kc - 1),
                )
        for st in range(ST):
            evac_fn(mc, st, pss[st])


def _v_phase(tc, pools, st, xt, wv_sb):
    """v projection for one sub-tile, token-partitioned (x is the
    stationary), ts-outer so only 2 PSUM banks are live. These dense
    matmuls are the PE gap filler under the softmax chains."""
    nc = tc.nc
    v_pool, ps_pool = pools["v"], pools["ps"]
    v_sb = v_pool.tile([128, NCH, NCH, 128], BF16, tag=f"v{st}", bufs=1,
                       name=f"v{st}")
    v2 = v_sb[:].rearrange("p a b c -> p (a b c)")
    for ts in range(NCH):
        ps = ps_pool.tile([128, TILE], FP32, tag=f"p{ts % 2}", bufs=2,
                          name="psv")
        for kc in range(NCH):
            nc.tensor.matmul(
                ps[:],
                lhsT=xt[:, st, kc, 128 * ts:128 * (ts + 1)],
                rhs=wv_sb[:, kc, :],
                start=(kc == 0), stop=(kc == NCH - 1),
            )
        if ts % 2 == 0:
            nc.scalar.copy(v2[:, 512 * ts:512 * (ts + 1)], ps[:])
        else:
            nc.vector.tensor_copy(v2[:, 512 * ts:512 * (ts + 1)], ps[:])
    return v_sb


def _scores_phase(tc, pools, axis, st, q_sb, kz, tml_sb, tmr_sb):
    """S + softmax chain for one sub-tile; returns the two abd tiles
    (block-diagonal normalized A^T) the AV phase will consume."""
    nc = tc.nc
    a_pool, sm_pool, ps_pool = pools["a"], pools["sm"], pools["ps"]
    GW = NH * 32
    abds = []
    for gg in range(2):
        sps = ps_pool.tile([128, 2 * GW], FP32, tag=f"p{2 + gg}", bufs=2,
                           name="sps")
        base = axis == "t"
        if base:
            nc.tensor.matmul(
                sps[:], lhsT=tml_sb[:], rhs=tmr_sb[:],
                start=True, stop=False, skip_group_check=True,
            )
        nmm = 32
        i_mm = 0
        for gh in range(2):
            g = 2 * gg + gh
            for c in range(NCH):
                for j in range(4):
                    qcol = (g * 4 + j) * 32
                    i_mm += 1
                    nc.tensor.matmul(
                        sps[32 * j:32 * (j + 1),
                            gh * GW + 2 * c * 32:gh * GW + (2 * c + 2) * 32],
                        lhsT=q_sb[:, st, c, qcol:qcol + 32],
                        rhs=kz[:, :, c,
                               (g * 4 + j) * 32:(g * 4 + j) * 32 + 32],
                        tile_position=(0, 32 * j),
                        start=(not base),
                        stop=(base and i_mm == nmm),
                        skip_group_check=True,
                    )
        # softmax over k (free axis)
        a_sb = a_pool.tile([128, 2 * GW], BF16, tag="a")
        nc.scalar.activation(a_sb[:], sps[:],
                             mybir.ActivationFunctionType.Exp)
        a3 = a_sb[:].rearrange("p (n k) -> p n k", n=2 * NH)
        sums = sm_pool.tile([128, 2 * NH], FP32, tag="sums")
        nc.vector.tensor_reduce(
            sums[:], a3, axis=mybir.AxisListType.X, op=mybir.AluOpType.add
        )
        recip = sm_pool.tile([128, 2 * NH], FP32, tag="recip")
        nc.vector.reciprocal(recip[:], sums[:])
        nc.gpsimd.tensor_tensor(
            a3, a3,
            recip[:].unsqueeze(2).broadcast_to((128, 2 * NH, 32)),
            mybir.AluOpType.mult,
        )
        at_sb = a_pool.tile([128, 2 * GW], BF16, tag="at")
        nc.vector.transpose(at_sb[:], a_sb[:])
        abd = pools["abd"][tc._abd_flip]
        tc._abd_flip = (tc._abd_flip + 1) % len(pools["abd"])
        for j in range(4):
            nc.vector.tensor_copy(
                abd[32 * j:32 * (j + 1), 512 * j:512 * (j + 1)],
                at_sb[32 * j:32 * (j + 1), :],
            )
        abds.append(abd)
    return abds


def _av_phase(tc, pools, st, v_sb, abds, ot_sb):
    """O^T = V^T @ A_bd for one sub-tile, chunk-outer."""
    nc = tc.nc
    ps_pool = pools["ps"]
    GW = NH * 32
    for c in range(NCH):
        otp = ps_pool.tile([128, TILE], FP32, name="otp",
                           tag=f"p{c % 2}", bufs=2)
        for g in range(4):
            gh = g % 2
            abd4 = abds[g // 2][:].rearrange("p (j x) -> p j x", j=4)
            for p in range(2):
                s0 = gh * GW + (2 * c + p) * 32
                nc.tensor.matmul(
                    otp[64 * p:64 * (p + 1), g * 128:(g + 1) * 128],
                    lhsT=v_sb[:, g, c, 64 * p:64 * (p + 1)],
                    rhs=abd4[:, :, s0:s0 + 32],
                    tile_position=(0, 64 * p),
                )
        if c % 2 == 0:
            nc.scalar.copy(ot_sb[:, st, c, :], otp[:])
        else:
            nc.vector.tensor_copy(ot_sb[:, st, c, :], otp[:])


def _load_xt(tc, pools, x_ap, sup):
    nc = tc.nc
    xt = pools["xt"].tile([128, ST, NCH, TILE], BF16, tag="xt")
    for st in range(ST):
        for kc in range(NCH):
            nc.sync.dma_start(
                xt[:, st, kc, :],
                x_ap[128 * kc:128 * (kc + 1),
                     (sup * ST + st) * TILE:(sup * ST + st + 1) * TILE])
    return xt


def _q_phase(tc, pools, xt, wq_sb):
    nc = tc.nc
    q_sb = pools["qk"].tile([128, ST, NCH, TILE], BF16, tag="q", bufs=2)

    def q_evac(mc, st, ps):
        if st % 2 == 0:
            nc.scalar.copy(q_sb[:, st, mc, :], ps[:])
        else:
            nc.vector.tensor_copy(q_sb[:, st, mc, :], ps[:])

    _proj_phase(tc, pools["ps"], NCH, NCH,
                lambda mc, kc: wq_sb[:, kc, 128 * mc:128 * (mc + 1)],
                lambda st, kc: xt[:, st, kc, :], q_evac)
    return q_sb


def _build_pass(tc, pools, axis, x_ap, w_aps, y_ap, tml_sb, tmr_sb,
                kz_tiles, preloaded=None, next_prologue=None):
    """One axial pass over the core's 8192 owned tokens; writes its own
    bf16 y output in the pass-local token order (host sums the three)."""
    nc = tc.nc
    wq_sb, wk_sb, wv_sb, wo_sb = w_aps
    ot_pool, y_pool, ps_pool = pools["ot"], pools["y"], pools["ps"]

    def load_xt(sup):
        return _load_xt(tc, pools, x_ap, sup)

    def q_phase(xt):
        return _q_phase(tc, pools, xt, wq_sb)

    def k_phase(xt):
        def k_evac(mc, st, ps):
            kz = kz_tiles[st]
            if mc < 2:
                nc.scalar.copy(kz[0:64, 0, mc, :], ps[0:64, :])
                nc.scalar.copy(kz[64:128, 1, mc, :], ps[64:128, :])
            else:
                nc.vector.tensor_copy(kz[0:64, 0, mc, :], ps[0:64, :])
                nc.vector.tensor_copy(kz[64:128, 1, mc, :], ps[64:128, :])

        _proj_phase(tc, ps_pool, NCH, NCH,
                    lambda mc, kc: wk_sb[:, kc, 128 * mc:128 * (mc + 1)],
                    lambda st, kc: xt[:, st, kc, :], k_evac)

    def o_phase(sup, ot_sb):
        it0 = sup * ST

        def y_evac(mc, st, ps):
            it = it0 + st
            cs = slice(128 * mc, 128 * (mc + 1))
            y_sb = y_pool.tile([128, TILE], BF16, tag="y_sb", bufs=4)
            if mc % 2 == 0:
                nc.scalar.copy(y_sb[:], ps[:])
            else:
                nc.vector.tensor_copy(y_sb[:], ps[:])
            nc.scalar.dma_start(y_ap[cs, it * TILE:(it + 1) * TILE],
                                y_sb[:])

        _proj_phase(tc, ps_pool, NCH, NCH,
                    lambda mc, kc: wo_sb[:, kc, 128 * mc:128 * (mc + 1)],
                    lambda st, kc: ot_sb[:, st, kc, :], y_evac)

    # ---- pass body with PE-stream interleave: scores run 2 sub-tiles
    # ahead of AV (the softmax chain of st hides under scores of st+1);
    # the next super-tile's q projection fills the last chain's gap, and
    # at the pass end the NEXT pass's first q projection does (prologue).
    if preloaded is not None:
        xt, q_sb = preloaded
    else:
        xt = load_xt(0)
        q_sb = q_phase(xt)
    k_phase(xt)
    next_pre = None
    for sup in range(NSUP):
        ot_sb = ot_pool.tile([128, ST, NCH, TILE], BF16, tag="ot", bufs=1)
        xt_next = q_next = None
        v_tiles = {0: _v_phase(tc, pools, 0, xt, wv_sb)}
        abds_by_st = {}
        for st in range(ST):
            abds_by_st[st] = _scores_phase(tc, pools, axis, st, q_sb,
                                           kz_tiles[st], tml_sb, tmr_sb)
            if st < ST - 1:
                v_tiles[st + 1] = _v_phase(tc, pools, st + 1, xt, wv_sb)
            if st >= 1:
                _av_phase(tc, pools, st - 1, v_tiles.pop(st - 1),
                          abds_by_st.pop(st - 1), ot_sb)
        if sup + 1 < NSUP:
            xt_next = load_xt(sup + 1)
            q_next = q_phase(xt_next)
        elif next_prologue is not None:
            next_pre = next_prologue()
        _av_phase(tc, pools, ST - 1, v_tiles.pop(ST - 1),
                  abds_by_st.pop(ST - 1), ot_sb)
        o_phase(sup, ot_sb)
        if sup + 1 < NSUP:
            k_phase(xt_next)
            xt, q_sb = xt_next, q_next
    return next_pre


def build_program():
    """Build + compile the SPMD bass program (same program on all 8 cores)."""
    nc = bacc.Bacc(
        "TRN2", target_bir_lowering=False, debug=False,
        enable_asserts=False, num_devices=N_CORES,
    )

    def din(name, shape, dt=BF16):
        return nc.dram_tensor(name, shape, dt, kind="ExternalInput").ap()

    x_in = {ax: din(f"x_{ax}", (C, TOK_LOCAL)) for ax in ("w", "t", "h")}
    w_in = {}
    for ax in ("w", "t", "h"):
        for nm in ("wq", "wk", "wv", "wo"):
            w_in[f"{nm}_{ax}"] = din(f"{nm}_{ax}", (C, C))
    tml_in = din("tml", (2, 128))
    tmr_in = din("tmr", (2, 512))
    y_aps = {ax: nc.dram_tensor(f"y_{ax}", (C, TOK_LOCAL), BF16,
                                kind="ExternalOutput").ap()
             for ax in ("w", "t", "h")}

    with tile.TileContext(nc) as tc:
        with contextlib.ExitStack() as ctx:
            xt_pool = ctx.enter_context(tc.tile_pool(name="xt", bufs=2))
            w_pool = ctx.enter_context(tc.tile_pool(name="wts", bufs=1))
            qk_pool = ctx.enter_context(tc.tile_pool(name="qk", bufs=2))
            v_pool = ctx.enter_context(tc.tile_pool(name="v", bufs=1))
            a_pool = ctx.enter_context(tc.tile_pool(name="a", bufs=3))
            sm_pool = ctx.enter_context(tc.tile_pool(name="sm", bufs=3))
            ot_pool = ctx.enter_context(tc.tile_pool(name="ot", bufs=1))
            y_pool = ctx.enter_context(tc.tile_pool(name="y", bufs=2))
            ps_pool = ctx.enter_context(tc.tile_pool(name="ps", bufs=1,
                                                     space="PSUM"))
            const_pool = ctx.enter_context(tc.tile_pool(name="const", bufs=1))

            # constants
            tml_sb = const_pool.tile([2, 128], BF16)
            nc.sync.dma_start(tml_sb[:], tml_in[:])
            tmr_sb = const_pool.tile([2, 512], BF16)
            nc.sync.dma_start(tmr_sb[:], tmr_in[:])

            # persistent block-diagonal A^T buffers and parity-split k
            # buffers (one per sub-tile), zeroed once
            abd_tiles = []
            for i in range(4):
                t = const_pool.tile([128, 4 * 512], BF16, name=f"abd{i}")
                nc.gpsimd.memset(t[:], 0.0)
                abd_tiles.append(t)
            tc._abd_flip = 0
            kz_tiles = []
            for i in range(ST):
                t = const_pool.tile([128, 2, NCH, TILE], BF16, name=f"kz{i}")
                nc.gpsimd.memset(t[:], 0.0)
                kz_tiles.append(t)

            pools = {"xt": xt_pool, "qk": qk_pool, "v": v_pool,
                     "a": a_pool, "sm": sm_pool, "ot": ot_pool,
                     "y": y_pool, "ps": ps_pool, "abd": abd_tiles}

            def load_w_one(ax, nm):
                wt = w_pool.tile([128, NCH, C], BF16, tag=nm, name=nm)
                for kc in range(NCH):
                    nc.sync.dma_start(
                        wt[:, kc, :],
                        w_in[f"{nm}_{ax}"][128 * kc:128 * (kc + 1), :],
                    )
                return wt

            axes = ("w", "t", "h")

            def make_prologue(nxt_ax):
                # emitted at the previous pass's last softmax gap: load
                # the next pass's wq + first x super-tile and emit its
                # q projection (independent PE work that fills the gap)
                def prologue():
                    wq_n = load_w_one(nxt_ax, "wq")
                    xt0 = _load_xt(tc, pools, x_in[nxt_ax], 0)
                    q0 = _q_phase(tc, pools, xt0, wq_n)
                    return (wq_n, xt0, q0)
                return prologue

            pre = None
            for i, ax in enumerate(axes):
                if pre is None:
                    wq_sb = load_w_one(ax, "wq")
                    xt0 = _load_xt(tc, pools, x_in[ax], 0)
                    q0 = _q_phase(tc, pools, xt0, wq_sb)
                else:
                    wq_sb, xt0, q0 = pre
                w_aps = [wq_sb] + [load_w_one(ax, nm)
                                   for nm in ("wk", "wv", "wo")]
                nxt = (make_prologue(axes[i + 1])
                       if i + 1 < len(axes) else None)
                pre = _build_pass(tc, pools, ax, x_in[ax], w_aps, y_aps[ax],
                                  tml_sb, tmr_sb, kz_tiles,
                                  preloaded=(xt0, q0), next_prologue=nxt)

    nc.compile()
    return nc


_PROGRAM = None


def _get_program():
    global _PROGRAM
    if _PROGRAM is None:
        _PROGRAM = build_program()
    return _PROGRAM


def make_in_maps(inputs):
    """Host-side shard + layout prep: per-core input dicts."""
    global _BIAS
    x = np.asarray(inputs["x"], np.float32)          # (B, C, T, H, W)
    scale = 1.0 / np.sqrt(D)
    _BIAS = (np.asarray(inputs["bo_w"], np.float32)
             + np.asarray(inputs["bo_h"], np.float32)
             + np.asarray(inputs["bo_t"], np.float32)).reshape(C, 1, 1, 1)

    weights = {}
    for ax in ("w", "t", "h"):
        for nm in ("wq", "wk", "wv", "wo"):
            wm = np.asarray(inputs[f"{nm}_{ax}"], np.float32)
            if nm == "wq":
                wm = wm * scale
            # lhsT layout: (C_in, C_out) = W.T
            weights[f"{nm}_{ax}"] = np.ascontiguousarray(wm.T).astype(BF16_NP)
    # rank-2 additive cross-fiber mask for the t-pass:
    # S += tml.T @ tmr with tml one-hot on the query fiber and tmr = -60 on
    # cross-fiber key columns
    p = np.arange(128) % 32
    tml = np.stack([(p // 16) == e for e in range(2)]).astype(BF16_NP)
    f = np.arange(512) % 32
    tmr = np.stack([np.where((f // 16) != e, -60.0, 0.0) for e in range(2)]
                   ).astype(BF16_NP)

    in_maps = []
    for core in range(N_CORES):
        b, j = divmod(core, 2)
        xb = x[b]                                    # (C, T, H, W)
        xw = xb[:, :, HL * j:HL * (j + 1), :]        # (C, T, HL, W) w-fastest
        xt = np.transpose(xw, (0, 2, 3, 1))          # (C, HL, W, T) t-fastest
        xh = np.transpose(xb, (0, 1, 3, 2))[:, :, WL * j:WL * (j + 1), :]
        # xh: (C, T, WL, H) h-fastest
        m = {
            "x_w": np.ascontiguousarray(xw).reshape(C, TOK_LOCAL).astype(BF16_NP),
            "x_t": np.ascontiguousarray(xt).reshape(C, TOK_LOCAL).astype(BF16_NP),
            "x_h": np.ascontiguousarray(xh).reshape(C, TOK_LOCAL).astype(BF16_NP),
            "tml": tml, "tmr": tmr,
        }
        m.update(weights)
        in_maps.append(m)
    return in_maps


_BIAS = None


def assemble_output(results):
    """Gather per-core y_w/y_t/y_h into (B, C, T, H, W) fp32."""
    out = np.empty((B, C, T, H, W), np.float32)
    bias = (_BIAS if _BIAS is not None else 0.0)
    for b in range(B):
        for j in range(2):
            core = 2 * b + j
            hs = slice(HL * j, HL * (j + 1))
            yw = np.asarray(results[core]["y_w"]).astype(np.float32)
            yt = np.asarray(results[core]["y_t"]).astype(np.float32)
            out[b, :, :, hs, :] = (
                yw.reshape(C, T, HL, W)
                + yt.reshape(C, HL, W, T).transpose(0, 3, 1, 2)
                + bias)
        for j in range(2):
            core = 2 * b + j
            yh = np.asarray(results[core]["y_h"]).astype(np.float32)
            out[b, :, :, :, WL * j:WL * (j + 1)] += yh.reshape(
                C, T, WL, H).transpose(0, 1, 3, 2)
    return out


_RUNNER = None


def _get_runner():
    """Build the sharded PJRT callable once; reuse across kernel() calls."""
    global _RUNNER
    if _RUNNER is not None:
        return _RUNNER
    import jax
    from jax.sharding import Mesh, PartitionSpec
    from jax.experimental.shard_map import shard_map
    from concourse import bass2jax

    nc = _get_program()
    bass2jax.install_neuronx_cc_hook()
    partition_name = (nc.partition_id_tensor.name
                      if nc.partition_id_tensor else None)
    in_names, out_names, out_avals, zero_outs = [], [], [], []
    for alloc in nc.m.functions[0].allocations:
        if not isinstance(alloc, mybir.MemoryLocationSet):
            continue
        name = alloc.memorylocations[0].name
        if alloc.kind == "ExternalInput":
            if name != partition_name:
                in_names.append(name)
        elif alloc.kind == "ExternalOutput":
            out_names.append(name)
            shape = tuple(alloc.tensor_shape)
            dtype = mybir.dt.np(alloc.dtype)
            out_avals.append(jax.core.ShapedArray(shape, dtype))
            zero_outs.append(np.zeros((N_CORES * shape[0], *shape[1:]), dtype))
    n_params = len(in_names)
    all_in_names = list(in_names) + out_names
    if partition_name is not None:
        all_in_names.append(partition_name)

    def _body(*args):
        operands = list(args)
        if partition_name is not None:
            operands.append(bass2jax.partition_id_tensor())
        return tuple(bass2jax._bass_exec_p.bind(
            *operands,
            out_avals=tuple(out_avals),
            in_names=tuple(all_in_names),
            out_names=tuple(out_names),
            lowering_input_output_aliases=(),
            sim_require_finite=True,
            sim_require_nnan=True,
            nc=nc,
        ))

    devices = jax.devices()[:N_CORES]
    mesh = Mesh(np.asarray(devices), ("core",))
    in_specs = (PartitionSpec("core"),) * (n_params + len(out_names))
    out_specs = (PartitionSpec("core"),) * len(out_names)
    fn = jax.jit(shard_map(_body, mesh=mesh, in_specs=in_specs,
                           out_specs=out_specs, check_rep=False))

    def run(in_maps):
        concat_in = [
            np.concatenate([np.asarray(in_maps[c][nm]) for c in range(N_CORES)],
                           axis=0)
            for nm in in_names
        ]
        outs = fn(*concat_in, *zero_outs)
        return [
            {nm: np.asarray(outs[i]).reshape(N_CORES, *out_avals[i].shape)[c]
             for i, nm in enumerate(out_names)}
            for c in range(N_CORES)
        ]

    _RUNNER = run
    return run


def kernel(**inputs) -> np.ndarray:
    run = _get_runner()
    in_maps = make_in_maps(inputs)
    return assemble_output(run(in_maps))
